# revision 32
# baseline (speedup 1.0000x reference)
"""Trainium2 Bass kernel for 8-head MultiHeadAttention (B=2, S=4096, E=512).

Sharding: 8 cores = 2 batches x 4 query-row chunks of 1024. Each core computes
all 8 heads for its (batch, q-range). Structure:
  - QK^T scores built transposed ([k partitions, q free]) as in the baseline.
  - softmax exp split across three engines: ACT (exact table exp) plus DVE and
    GPSIMD using a single-instruction Schraudolph bit-trick (int16 write
    bitcast to bf16), all masked multiplicatively afterward on DVE.
  - The attention-value matmul uses pt blocks as the stationary operand so the
    output lands as ctx[q partitions, d free] with a ones-column denominator:
    full 128-partition output halves the PE row count vs the [d, q] layout.
  - Wv is folded into Wo on the host (Wo' = Wo @ blockdiag(Wv)) so no V
    projection runs on device; normalization is a per-partition reciprocal
    plus a free-dim broadcast multiply straight into concat staging.
  - concat [q, e] is flipped to [e, q] via PE transposes against a host
    identity, then the output projection streams q rows per 128-q chunk.
"""
import sys
for _p in ('/root/.axon_site/_ro/trn_rl_repo', '/opt/trn_rl_repo'):
    if _p not in sys.path:
        sys.path.append(_p)

import numpy as np
import ml_dtypes

import concourse.bass as bass
import concourse.tile as tile
from concourse import bacc, mybir
from concourse import bass_utils

F32 = mybir.dt.float32
BF16 = mybir.dt.bfloat16
I16 = mybir.dt.int16
AF = mybir.ActivationFunctionType
ALU = mybir.AluOpType

N_CORES = 8
B, S, E, H, DH = 2, 4096, 512, 8, 64
QLEN = S // 4          # 1024 q rows per core
KC = S // 128          # 32 k chunks
QW = QLEN // 512       # 2 q windows of 512

# Schraudolph exp-as-bf16-bits: int16(x*EXPA + EXPB) bitcast bf16 ~ exp(x/8)
LOG2E = 1.4426950408889634
EXPA = 128.0 * LOG2E / 8.0
EXPB = 128.0 * (127.0 - 0.05735) + 0.5  # +0.5 compensates trunc-toward-zero

# exp engine split per 32-chunk window: 'a'=ACT exact, 'p'=Pool, 'v'=DVE.
# Pool exps sit at kc>=8 so Pool is idle at window boundaries and can run
# the normalize ops of the previous window immediately.
EXP_ENG = {}
_pool_kcs = {8, 10, 12, 14, 16, 18, 20, 22, 24, 26, 28, 30}
_dve_kcs = {31}
for _kc in range(KC):
    EXP_ENG[_kc] = 'p' if _kc in _pool_kcs else ('v' if _kc in _dve_kcs else 'a')

_CACHE = {}


def _build_module():
    nc = bacc.Bacc("TRN2", target_bir_lowering=False, debug=False,
                   enable_asserts=True, num_devices=N_CORES)

    xqT = nc.dram_tensor("xqT", [E, QLEN], BF16, kind="ExternalInput").ap()
    xkT = nc.dram_tensor("xkT", [E, S], BF16, kind="ExternalInput").ap()
    valp = nc.dram_tensor("valp", [S, H * 65], BF16, kind="ExternalInput").ap()
    maskT = nc.dram_tensor("maskT", [S, QLEN], BF16, kind="ExternalInput").ap()
    ident = nc.dram_tensor("ident", [128, 128], BF16, kind="ExternalInput").ap()
    wqT = nc.dram_tensor("wqT", [DH, DH], BF16, kind="ExternalInput").ap()
    wkT = nc.dram_tensor("wkT", [DH, DH], BF16, kind="ExternalInput").ap()
    woT = nc.dram_tensor("woT", [E, E], BF16, kind="ExternalInput").ap()
    bo_b = nc.dram_tensor("bo_b", [128, E], F32, kind="ExternalInput").ap()
    out = nc.dram_tensor("out", [QLEN, E], F32, kind="ExternalOutput").ap()

    with tile.TileContext(nc) as tc:
        _emit(tc, nc, xqT, xkT, valp, maskT, ident, wqT, wkT, woT, bo_b, out)

    nc.compile()
    return nc


def _emit(tc, nc, xqT, xkT, valp, maskT, ident, wqT, wkT, woT, bo_b, out):
    from contextlib import ExitStack
    ctx = ExitStack()
    const = ctx.enter_context(tc.tile_pool(name="const", bufs=1))
    kpool = ctx.enter_context(tc.tile_pool(name="kproj", bufs=1))
    qpool = ctx.enter_context(tc.tile_pool(name="qproj", bufs=2))
    xkst = ctx.enter_context(tc.tile_pool(name="xkst", bufs=2))
    ppool = ctx.enter_context(tc.tile_pool(name="p", bufs=10))
    rcpool = ctx.enter_context(tc.tile_pool(name="rc", bufs=2))
    ospool = ctx.enter_context(tc.tile_pool(name="osb", bufs=2))
    psp = ctx.enter_context(tc.tile_pool(name="psp", bufs=3, space="PSUM"))
    uacc = ctx.enter_context(tc.tile_pool(name="uacc", bufs=1, space="PSUM"))

    def pstile(nm):
        """Scratch PSUM [128, 512] carved from the shared ps rotation."""
        t = psp.tile([128, 1024], F32, tag="ps", name=nm)
        return t[:, 0:512]

    # ---- resident mask tiles, 4 k-chunks per tile (loaded once, batched
    # DMAs: HWDGE descriptor generation is ~630ns per dma_start, so window 0
    # can't afford one DMA per 128-row chunk) ----
    mask_res = [const.tile([128, 4 * QLEN], BF16, tag=f"mk{c}", name=f"mk{c}")
                for c in range(KC // 4)]

    def mask_ap(kc, qw):
        t = mask_res[kc // 4]
        return t[:, (kc % 4) * QLEN + qw * 512:(kc % 4) * QLEN + qw * 512 + 512]

    def load_masks():
        for c in range(KC // 4):
            dst = mask_res[c].rearrange("p (c q) -> p c q", c=4)
            src = bass.AP(tensor=maskT.tensor, offset=c * 512 * QLEN,
                          ap=[[QLEN, 128], [128 * QLEN, 4], [1, QLEN]])
            nc.sync.dma_start(dst, src)

    # ---- constants (wq/wk immediately; heavy/late consts after proj0 loads)
    wq_sb = const.tile([DH, DH], BF16, tag="wq")
    nc.gpsimd.dma_start(wq_sb, wqT)
    wk_sb = const.tile([DH, DH], BF16, tag="wk")
    nc.gpsimd.dma_start(wk_sb, wkT)
    ident_sb = const.tile([128, 128], BF16, tag="ident")
    nc.gpsimd.dma_start(ident_sb, ident)
    wo_sb = []
    for pc in range(4):
        wo_sb.append(const.tile([128, E], BF16, tag=f"wo{pc}", name=f"wo{pc}"))
    bo_sb = const.tile([128, E], F32, tag="bo")

    def load_late_consts():
        for pc in range(4):
            nc.gpsimd.dma_start(wo_sb[pc], woT[pc * 128:(pc + 1) * 128, :])
        nc.gpsimd.dma_start(bo_sb, bo_b)

    # valp, 4 k-chunks per tile, batched DMAs on the ACT HWDGE queue
    VW = H * 65
    valp_t = [const.tile([128, 4 * VW], BF16, tag=f"vp{c}", name=f"vp{c}")
              for c in range(KC // 4)]

    def load_valp():
        for c in range(KC // 4):
            dst = valp_t[c].rearrange("p (c v) -> p c v", c=4)
            src = bass.AP(tensor=valp.tensor, offset=c * 512 * VW,
                          ap=[[VW, 128], [128 * VW, 4], [1, VW]])
            nc.scalar.dma_start(dst, src)

    def valp_ap(kc, h):
        t = valp_t[kc // 4]
        return t[:, (kc % 4) * VW + h * 65:(kc % 4) * VW + h * 65 + 65]

    # concat staging [q 128, E] bf16, all 8 q-chunks in one tile so the
    # normalize divide can hit all 4 qsubs of a window in one instruction
    conc_sb = const.tile([128, 8 * E], BF16, tag="cq")
    # transposed concat [e, q] for the out-projection lhsT: 4 pc-blocks x QLEN
    ct_sb = const.tile([128, 4 * QLEN], BF16, tag="ct")

    kproj_sb = [None] * 4
    qproj_sb = [None] * 4
    xs = {}

    def proj_load(pair, only_first=False, only_second=False):
        # pair 0 is startup-critical (sync + scalar HWDGE); pair 1 rides the
        # gpsimd SWDGE queue before Pool's exp work begins; pairs 2-3 use the
        # scalar queue, which is empty by then.
        qk = nc.sync if pair == 0 else (nc.gpsimd if pair == 1 else nc.scalar)
        qq = nc.scalar if pair == 0 else (nc.gpsimd if pair == 1 else nc.scalar)
        if not only_second:
            kproj_sb[pair] = kpool.tile([128, S], BF16, tag=f"kp{pair}",
                                        name=f"kp{pair}")
            qproj_sb[pair] = qpool.tile([128, QLEN], BF16, tag="qp",
                                        name=f"qp{pair}")
            xk0 = xkst.tile([DH, S], BF16, tag="xk", name=f"xk0_{pair}")
            qk.dma_start(xk0, xkT[(2 * pair) * DH:(2 * pair + 1) * DH, :])
            xq0 = xkst.tile([DH, QLEN], BF16, tag="xq", name=f"xq0_{pair}")
            qq.dma_start(xq0, xqT[(2 * pair) * DH:(2 * pair + 1) * DH, :])
            xs[pair] = (xk0, None, xq0, None)
            if only_first:
                return
        xk0, _, xq0, _ = xs[pair]
        xk1 = xkst.tile([DH, S], BF16, tag="xk", name=f"xk1_{pair}")
        qk.dma_start(xk1, xkT[(2 * pair + 1) * DH:(2 * pair + 2) * DH, :])
        xq1 = xkst.tile([DH, QLEN], BF16, tag="xq", name=f"xq1_{pair}")
        qq.dma_start(xq1, xqT[(2 * pair + 1) * DH:(2 * pair + 2) * DH, :])
        xs[pair] = (xk0, xk1, xq0, xq1)

    def proj_chunks(pair, fast_start=False):
        """Closures: 8 kproj chunks + 2 qproj chunks. Copies run on ACT.
        xs[pair] is read lazily so loads may be staged in two steps."""
        kp_sb = kproj_sb[pair]
        qp_sb = qproj_sb[pair]

        def half(dst, src_i, w, h2, nm):
            def go():
                t = pstile(nm)
                lo, hi = h2 * 64, (h2 + 1) * 64
                nc.tensor.matmul(t[lo:hi, :], lhsT=w,
                                 rhs=xs[pair][src_i][:, 0:512],
                                 start=True, stop=True)
                nc.scalar.copy(dst[lo:hi, 0:512], t[lo:hi, :])
            return go

        def kchunk(kc):
            def go():
                xk0, xk1 = xs[pair][0], xs[pair][1]
                kp = pstile(f"kpp{pair}_{kc}")
                nc.tensor.matmul(kp[0:64, :], lhsT=wk_sb,
                                 rhs=xk0[:, kc * 512:(kc + 1) * 512],
                                 start=True, stop=True)
                nc.tensor.matmul(kp[64:128, :], lhsT=wk_sb,
                                 rhs=xk1[:, kc * 512:(kc + 1) * 512],
                                 start=True, stop=True)
                nc.scalar.copy(kp_sb[:, kc * 512:(kc + 1) * 512], kp)
            return go

        def qchunk(qc):
            def go():
                xq0, xq1 = xs[pair][2], xs[pair][3]
                qp = pstile(f"qpp{pair}_{qc}")
                nc.tensor.matmul(qp[0:64, :], lhsT=wq_sb,
                                 rhs=xq0[:, qc * 512:(qc + 1) * 512],
                                 start=True, stop=True)
                nc.tensor.matmul(qp[64:128, :], lhsT=wq_sb,
                                 rhs=xq1[:, qc * 512:(qc + 1) * 512],
                                 start=True, stop=True)
                nc.scalar.copy(qp_sb[:, qc * 512:(qc + 1) * 512], qp)
            return go

        if fast_start:
            return ([half(kp_sb, 0, wk_sb, 0, "fk0"),
                     half(qp_sb, 2, wq_sb, 0, "fq0"),
                     half(kp_sb, 1, wk_sb, 1, "fk1"),
                     half(qp_sb, 3, wq_sb, 1, "fq1")]
                    + [kchunk(kc) for kc in range(1, 8)]
                    + [qchunk(qc) for qc in range(1, QW)])
        return ([kchunk(0), qchunk(0)] + [kchunk(kc) for kc in range(1, 8)]
                + [qchunk(qc) for qc in range(1, QW)])

    def attn(pair, qw, trickle=()):
        """One (head-pair, q-window): scores -> exp -> mask -> PV accumulate.

        Superseded by the global pipeline below; kept out of use."""
        raise NotImplementedError

    def finish_chunk(c):
        """Transpose concat chunk c into ct and run its out-projection."""
        def tr():
            ut = pstile(f"tr{c}")
            ut_bf = ut.bitcast(BF16)
            for pc in range(4):
                nc.tensor.transpose(ut_bf[:, pc * 128:(pc + 1) * 128],
                                    conc_sb[:, c * E + pc * 128:
                                            c * E + (pc + 1) * 128],
                                    ident_sb)
            ctv = bass.AP(tensor=ct_sb.tensor, offset=ct_sb.offset + c * 128,
                          ap=[ct_sb.ap[0], [QLEN, 4], [1, 128]])
            nc.vector.tensor_copy(ctv, ut_bf[:, 0:512])

        def op():
            o = pstile(f"op{c}")
            for pc in range(4):
                nc.tensor.matmul(o,
                                 lhsT=ct_sb[:, pc * QLEN + c * 128:
                                            pc * QLEN + (c + 1) * 128],
                                 rhs=wo_sb[pc],
                                 start=(pc == 0), stop=(pc == 3))
            osb = ospool.tile([128, E], F32, tag="osb", name=f"osb{c}")
            nc.vector.scalar_tensor_tensor(osb, o, 1.0, bo_sb,
                                           ALU.mult, ALU.add)
            nc.sync.dma_start(out[c * 128:(c + 1) * 128, :], osb)
        return [tr, op]

    # ---- global attention pipeline -------------------------------------
    # One continuous stream of 256 chunks (8 windows x 32 k-chunks); scores/
    # exp/mask run SKEW chunks ahead of the PV accumulation so neither PE nor
    # the elementwise engines ever wait on the in-flight chunk, including
    # across window boundaries.
    from collections import deque
    WINDOWS = [(p, w) for p in range(4) for w in range(QW)]
    SKEW = 8
    NG = len(WINDOWS) * KC
    Umap = {}
    pts = {}

    def emit_scores(g):
        w, kc = divmod(g, KC)
        pair, qw = WINDOWS[w]
        kp_sb = kproj_sb[pair]
        qp_sb = qproj_sb[pair]
        ps = psp.tile([128, 1024], F32, tag="ps", name=f"ps{w}_{kc}")
        nc.tensor.matmul(
            ps[:, 0:512],
            lhsT=kp_sb[0:64, kc * 128:(kc + 1) * 128],
            rhs=qp_sb[0:64, qw * 512:(qw + 1) * 512],
            start=True, stop=True)
        nc.tensor.matmul(
            ps[:, 512:1024],
            lhsT=kp_sb[64:128, kc * 128:(kc + 1) * 128],
            rhs=qp_sb[64:128, qw * 512:(qw + 1) * 512],
            start=True, stop=True)
        pt = ppool.tile([128, 1024], BF16, tag="pt", name=f"pt{w}_{kc}")
        pts[g] = pt
        eng = EXP_ENG[kc]
        if eng == 'a':
            nc.scalar.activation(pt, ps, AF.Exp, bias=0.0, scale=0.125)
        else:
            e = nc.vector if eng == 'v' else nc.gpsimd
            e.tensor_scalar(pt.bitcast(I16), ps, EXPA, EXPB,
                            ALU.mult, ALU.add)
        # mask multiply (DVE, bf16 2x), broadcast across the head pair
        ms = mask_ap(kc, qw)
        mb = bass.AP(tensor=ms.tensor, offset=ms.offset,
                     ap=[ms.ap[0], [0, 2], [1, 512]])
        pv = pt.rearrange("p (h q) -> p h q", h=2)
        nc.vector.tensor_mul(pv, pv, mb)

    def emit_norms(w, inline_fins=False):
        """Normalize window w's context into concat staging. Runs on Pool
        (idle at window boundaries) as a single divide per (head, qsub):
        out = ctx_cols / denominator_col, broadcast from PSUM. For the last
        window the finish work is emitted eagerly per q-chunk."""
        pair, qw = WINDOWS[w]
        U = Umap[w]
        for h2 in range(2):
            h = 2 * pair + h2
            uin = bass.AP(tensor=U[h2].tensor, offset=U[h2].offset,
                          ap=[U[h2].ap[0], [65, 4], [1, 64]])
            den = bass.AP(tensor=U[h2].tensor, offset=U[h2].offset + 64,
                          ap=[U[h2].ap[0], [65, 4], [0, 64]])
            co = bass.AP(tensor=conc_sb.tensor,
                         offset=conc_sb.offset + (qw * 4) * E + h * 64,
                         ap=[conc_sb.ap[0], [E, 4], [1, 64]])
            nc.gpsimd.scalar_tensor_tensor(co, uin, 1.0, den,
                                           ALU.mult, ALU.divide)
        if inline_fins:
            for qs in range(4):
                for work in finish_chunk(qw * 4 + qs):
                    work()

    def emit_pv(g):
        w, kc = divmod(g, KC)
        pair, qw = WINDOWS[w]
        if kc == 0:
            Umap[w] = [uacc.tile([128, 260], F32, tag=f"u{h2}",
                                 name=f"U{w}_{h2}")
                       for h2 in range(2)]
        U = Umap[w]
        pt = pts.pop(g)
        # One start=True per U bank marks the whole bank pending-zero; each
        # group's first write then zero-fills its own bytes.
        for h2 in range(2):
            h = 2 * pair + h2
            for qs in range(4):
                nc.tensor.matmul(
                    U[h2][:, qs * 65:qs * 65 + 65],
                    lhsT=pt[:, h2 * 512 + qs * 128:h2 * 512 + (qs + 1) * 128],
                    rhs=valp_ap(kc, h),
                    start=(kc == 0 and qs == 0), stop=(kc == KC - 1),
                    skip_group_check=True)
        if kc == KC - 1:
            emit_norms(w, inline_fins=(w == len(WINDOWS) - 1))

    # startup: xk0/xq0 first, then the first projection halves so their ACT
    # copies queue ahead of the bulk DMA configs, then everything else
    proj_load(0, only_first=True)
    fs = proj_chunks(0, fast_start=True)
    fs[0]()  # fk0 (k cols 0:512, both heads)
    fs[1]()  # fq0 (q window 0)
    proj_load(0, only_second=True)
    load_masks()
    load_valp()
    proj_load(1)
    load_late_consts()
    aux = deque(fs[2:])
    aux.extend(proj_chunks(1))
    for g in range(NG + SKEW):
        if g == 44:
            proj_load(2)
        elif g == 56:
            aux.extend(proj_chunks(2))
        elif g == 108:
            proj_load(3)
        elif g == 120:
            aux.extend(proj_chunks(3))
        if g < NG:
            emit_scores(g)
        if g >= SKEW:
            emit_pv(g - SKEW)
            wv, kcv = divmod(g - SKEW, KC)
            if kcv == KC - 1 and WINDOWS[wv] == (3, 0):
                for c in range(0, 4):
                    aux.extend(finish_chunk(c))
        if aux:
            aux.popleft()()
            if aux and g >= NG - 8:
                aux.popleft()()
    while aux:
        aux.popleft()()

    ctx.close()


def _prep_inputs(key, query, value, mask, Wq, Wk, Wv, Wo, bo):
    bf16 = ml_dtypes.bfloat16
    key = np.asarray(key, np.float32)
    query = np.asarray(query, np.float32)
    value = np.asarray(value, np.float32)
    mask = np.asarray(mask)
    Wv = np.asarray(Wv, np.float32)
    Wo = np.asarray(Wo, np.float32)
    # fold the V projection into the output projection:
    # concat_h(ctxraw_h @ Wv^T) @ Wo^T == concat_raw @ (Wo @ blockdiag(Wv))^T
    Wof = np.empty_like(Wo)
    for h in range(H):
        Wof[:, h * DH:(h + 1) * DH] = Wo[:, h * DH:(h + 1) * DH] @ Wv
    common = {
        "wqT": np.ascontiguousarray(np.asarray(Wq, np.float32).T).astype(bf16),
        "wkT": np.ascontiguousarray(np.asarray(Wk, np.float32).T).astype(bf16),
        "woT": np.ascontiguousarray(Wof.T).astype(bf16),
        "bo_b": np.ascontiguousarray(
            np.broadcast_to(np.asarray(bo, np.float32), (128, E))),
        "ident": np.eye(128, dtype=np.float32).astype(bf16),
    }
    maskT = np.ascontiguousarray(
        (mask[0, 0] != 0).astype(np.float32).T.astype(bf16))  # [k, q]
    per_b = {}
    for b in range(B):
        vp = np.ones((S, H, 65), np.float32)
        vp[:, :, :64] = value[b].reshape(S, H, DH)
        per_b[b] = {
            "xkT": np.ascontiguousarray(key[b].T).astype(bf16),
            "valp": np.ascontiguousarray(vp.reshape(S, H * 65).astype(bf16)),
            "qT": query[b].T,
        }
    in_maps = []
    for c in range(N_CORES):
        b, qs = c // 4, (c % 4) * QLEN
        in_maps.append({
            "xqT": np.ascontiguousarray(
                per_b[b]["qT"][:, qs:qs + QLEN]).astype(bf16),
            "xkT": per_b[b]["xkT"],
            "valp": per_b[b]["valp"],
            "maskT": np.ascontiguousarray(maskT[:, qs:qs + QLEN]),
            **common,
        })
    return in_maps


def get_module():
    if "nc" not in _CACHE:
        _CACHE["nc"] = _build_module()
    return _CACHE["nc"]


def kernel(key, query, value, mask, Wq, Wk, Wv, Wo, bo, **_):
    nc = get_module()
    in_maps = _prep_inputs(key, query, value, mask, Wq, Wk, Wv, Wo, bo)
    res = bass_utils.run_bass_kernel_spmd(
        nc, in_maps, core_ids=list(range(N_CORES)))
    full = np.empty((B, S, E), np.float32)
    for c in range(N_CORES):
        b, qs = c // 4, (c % 4) * QLEN
        full[b, qs:qs + QLEN, :] = res.results[c]["out"]
    return full


# revision 37
# speedup vs baseline: 1.0765x; 1.0765x over previous
"""Trainium2 Bass kernel for 8-head MultiHeadAttention (B=2, S=4096, E=512).

Sharding: 8 cores = 2 batches x 4 query-row chunks of 1024. Each core computes
all 8 heads for its (batch, q-range). Structure:
  - QK^T scores built transposed ([k partitions, q free]) as in the baseline.
  - softmax exp split across three engines: ACT (exact table exp) plus DVE and
    GPSIMD using a single-instruction Schraudolph bit-trick (int16 write
    bitcast to bf16), all masked multiplicatively afterward on DVE.
  - The attention-value matmul uses pt blocks as the stationary operand so the
    output lands as ctx[q partitions, d free] with a ones-column denominator:
    full 128-partition output halves the PE row count vs the [d, q] layout.
  - Wv is folded into Wo on the host (Wo' = Wo @ blockdiag(Wv)) so no V
    projection runs on device; normalization is a per-partition reciprocal
    plus a free-dim broadcast multiply straight into concat staging.
  - concat [q, e] is flipped to [e, q] via PE transposes against a host
    identity, then the output projection streams q rows per 128-q chunk.
"""
import sys
for _p in ('/root/.axon_site/_ro/trn_rl_repo', '/opt/trn_rl_repo'):
    if _p not in sys.path:
        sys.path.append(_p)

import numpy as np
import ml_dtypes

import concourse.bass as bass
import concourse.tile as tile
from concourse import bacc, mybir
from concourse import bass_utils

F32 = mybir.dt.float32
BF16 = mybir.dt.bfloat16
I16 = mybir.dt.int16
AF = mybir.ActivationFunctionType
ALU = mybir.AluOpType

N_CORES = 8
B, S, E, H, DH = 2, 4096, 512, 8, 64
QLEN = S // 4          # 1024 q rows per core
KC = S // 128          # 32 k chunks
QW = QLEN // 512       # 2 q windows of 512

# Schraudolph exp-as-bf16-bits: int16(x*EXPA + EXPB) bitcast bf16 ~ exp(x/8)
LOG2E = 1.4426950408889634
EXPA = 128.0 * LOG2E / 8.0
EXPB = 128.0 * (127.0 - 0.05735) + 0.5  # +0.5 compensates trunc-toward-zero

# exp engine assignment per half-chunk (each chunk's two head-halves exp on
# different engines in parallel): 'a'=ACT exact (612ns), 'p'=Pool bit-trick
# (850ns), 'v'=DVE bit-trick (594ns). Period 16 chunks: 17a/13p/2v keeps
# every engine under the PE chunk cadence in aggregate.
_EXP_PAT = [('a', 'p'), ('p', 'a'), ('a', 'a'), ('p', 'a'),
            ('a', 'p'), ('v', 'a'), ('p', 'a'), ('a', 'p'),
            ('p', 'a'), ('a', 'p'), ('a', 'a'), ('p', 'a'),
            ('a', 'p'), ('v', 'a'), ('p', 'a'), ('a', 'p')]

_CACHE = {}


def _build_module():
    nc = bacc.Bacc("TRN2", target_bir_lowering=False, debug=False,
                   enable_asserts=True, num_devices=N_CORES)

    xqT = nc.dram_tensor("xqT", [E, QLEN], BF16, kind="ExternalInput").ap()
    xkT = nc.dram_tensor("xkT", [E, S], BF16, kind="ExternalInput").ap()
    valp = nc.dram_tensor("valp", [S, H * 65], BF16, kind="ExternalInput").ap()
    maskT = nc.dram_tensor("maskT", [S, QLEN], BF16, kind="ExternalInput").ap()
    ident = nc.dram_tensor("ident", [128, 128], BF16, kind="ExternalInput").ap()
    wqT = nc.dram_tensor("wqT", [DH, DH], BF16, kind="ExternalInput").ap()
    wkT = nc.dram_tensor("wkT", [DH, DH], BF16, kind="ExternalInput").ap()
    woT = nc.dram_tensor("woT", [E, E], BF16, kind="ExternalInput").ap()
    bo_b = nc.dram_tensor("bo_b", [128, E], F32, kind="ExternalInput").ap()
    out = nc.dram_tensor("out", [QLEN, E], F32, kind="ExternalOutput").ap()

    with tile.TileContext(nc) as tc:
        _emit(tc, nc, xqT, xkT, valp, maskT, ident, wqT, wkT, woT, bo_b, out)

    nc.compile()
    return nc


def _emit(tc, nc, xqT, xkT, valp, maskT, ident, wqT, wkT, woT, bo_b, out):
    from contextlib import ExitStack
    ctx = ExitStack()
    const = ctx.enter_context(tc.tile_pool(name="const", bufs=1))
    kpool = ctx.enter_context(tc.tile_pool(name="kproj", bufs=1))
    qpool = ctx.enter_context(tc.tile_pool(name="qproj", bufs=2))
    xkst = ctx.enter_context(tc.tile_pool(name="xkst", bufs=2))
    ppool = ctx.enter_context(tc.tile_pool(name="p", bufs=10))
    rcpool = ctx.enter_context(tc.tile_pool(name="rc", bufs=2))
    ospool = ctx.enter_context(tc.tile_pool(name="osb", bufs=2))
    psp = ctx.enter_context(tc.tile_pool(name="psp", bufs=6, space="PSUM"))
    uacc = ctx.enter_context(tc.tile_pool(name="uacc", bufs=1, space="PSUM"))

    def pstile(nm):
        """Scratch PSUM [128, 512] from the shared ps rotation (one bank)."""
        return psp.tile([128, 512], F32, tag="ps", name=nm)

    # ---- resident mask tiles, 4 k-chunks per tile (loaded once, batched
    # DMAs: HWDGE descriptor generation is ~630ns per dma_start, so window 0
    # can't afford one DMA per 128-row chunk) ----
    mask_res = [const.tile([128, 4 * QLEN], BF16, tag=f"mk{c}", name=f"mk{c}")
                for c in range(KC // 4)]

    def mask_ap(kc, qw):
        t = mask_res[kc // 4]
        return t[:, (kc % 4) * QLEN + qw * 512:(kc % 4) * QLEN + qw * 512 + 512]

    def load_masks():
        for c in range(KC // 4):
            dst = mask_res[c].rearrange("p (c q) -> p c q", c=4)
            src = bass.AP(tensor=maskT.tensor, offset=c * 512 * QLEN,
                          ap=[[QLEN, 128], [128 * QLEN, 4], [1, QLEN]])
            nc.sync.dma_start(dst, src)

    # ---- constants (wq/wk immediately; heavy/late consts after proj0 loads)
    wq_sb = const.tile([DH, DH], BF16, tag="wq")
    nc.gpsimd.dma_start(wq_sb, wqT)
    wk_sb = const.tile([DH, DH], BF16, tag="wk")
    nc.gpsimd.dma_start(wk_sb, wkT)
    ident_sb = const.tile([128, 128], BF16, tag="ident")
    nc.gpsimd.dma_start(ident_sb, ident)
    wo_sb = []
    for pc in range(4):
        wo_sb.append(const.tile([128, E], BF16, tag=f"wo{pc}", name=f"wo{pc}"))
    bo_sb = const.tile([128, E], F32, tag="bo")

    def load_late_consts():
        for pc in range(4):
            nc.gpsimd.dma_start(wo_sb[pc], woT[pc * 128:(pc + 1) * 128, :])
        nc.gpsimd.dma_start(bo_sb, bo_b)

    # valp, 4 k-chunks per tile, batched DMAs on the ACT HWDGE queue
    VW = H * 65
    valp_t = [const.tile([128, 4 * VW], BF16, tag=f"vp{c}", name=f"vp{c}")
              for c in range(KC // 4)]

    def load_valp():
        for c in range(KC // 4):
            dst = valp_t[c].rearrange("p (c v) -> p c v", c=4)
            src = bass.AP(tensor=valp.tensor, offset=c * 512 * VW,
                          ap=[[VW, 128], [128 * VW, 4], [1, VW]])
            nc.scalar.dma_start(dst, src)

    def valp_ap(kc, h):
        t = valp_t[kc // 4]
        return t[:, (kc % 4) * VW + h * 65:(kc % 4) * VW + h * 65 + 65]

    # concat staging [q 128, E] bf16, all 8 q-chunks in one tile so the
    # normalize divide can hit all 4 qsubs of a window in one instruction
    conc_sb = const.tile([128, 8 * E], BF16, tag="cq")
    # transposed concat [e, q] for the out-projection lhsT: 4 pc-blocks x QLEN
    ct_sb = const.tile([128, 4 * QLEN], BF16, tag="ct")

    kproj_sb = [None] * 4
    qproj_sb = [None] * 4
    xs = {}

    def proj_load(pair, only_first=False, only_second=False):
        # pair 0 is startup-critical (sync + scalar HWDGE); pair 1 rides the
        # gpsimd SWDGE queue before Pool's exp work begins; pairs 2-3 use the
        # scalar queue, which is empty by then.
        qk = nc.sync if pair == 0 else (nc.gpsimd if pair == 1 else nc.scalar)
        qq = nc.scalar if pair == 0 else (nc.gpsimd if pair == 1 else nc.scalar)
        if not only_second:
            kproj_sb[pair] = kpool.tile([128, S], BF16, tag=f"kp{pair}",
                                        name=f"kp{pair}")
            qproj_sb[pair] = qpool.tile([128, QLEN], BF16, tag="qp",
                                        name=f"qp{pair}")
            xk0 = xkst.tile([DH, S], BF16, tag="xk", name=f"xk0_{pair}")
            qk.dma_start(xk0, xkT[(2 * pair) * DH:(2 * pair + 1) * DH, :])
            xq0 = xkst.tile([DH, QLEN], BF16, tag="xq", name=f"xq0_{pair}")
            qq.dma_start(xq0, xqT[(2 * pair) * DH:(2 * pair + 1) * DH, :])
            xs[pair] = (xk0, None, xq0, None)
            if only_first:
                return
        xk0, _, xq0, _ = xs[pair]
        xk1 = xkst.tile([DH, S], BF16, tag="xk", name=f"xk1_{pair}")
        qk.dma_start(xk1, xkT[(2 * pair + 1) * DH:(2 * pair + 2) * DH, :])
        xq1 = xkst.tile([DH, QLEN], BF16, tag="xq", name=f"xq1_{pair}")
        qq.dma_start(xq1, xqT[(2 * pair + 1) * DH:(2 * pair + 2) * DH, :])
        xs[pair] = (xk0, xk1, xq0, xq1)

    def proj_chunks(pair, fast_start=False):
        """Closures: 8 kproj chunks + 2 qproj chunks. Copies run on ACT.
        xs[pair] is read lazily so loads may be staged in two steps."""
        kp_sb = kproj_sb[pair]
        qp_sb = qproj_sb[pair]

        def half(dst, src_i, w, h2, nm):
            def go():
                t = pstile(nm)
                lo, hi = h2 * 64, (h2 + 1) * 64
                nc.tensor.matmul(t[lo:hi, :], lhsT=w,
                                 rhs=xs[pair][src_i][:, 0:512],
                                 start=True, stop=True)
                nc.scalar.copy(dst[lo:hi, 0:512], t[lo:hi, :])
            return go

        def kchunk(kc):
            def go():
                xk0, xk1 = xs[pair][0], xs[pair][1]
                kp = pstile(f"kpp{pair}_{kc}")
                nc.tensor.matmul(kp[0:64, :], lhsT=wk_sb,
                                 rhs=xk0[:, kc * 512:(kc + 1) * 512],
                                 start=True, stop=True)
                nc.tensor.matmul(kp[64:128, :], lhsT=wk_sb,
                                 rhs=xk1[:, kc * 512:(kc + 1) * 512],
                                 start=True, stop=True)
                nc.scalar.copy(kp_sb[:, kc * 512:(kc + 1) * 512], kp)
            return go

        def qchunk(qc):
            def go():
                xq0, xq1 = xs[pair][2], xs[pair][3]
                qp = pstile(f"qpp{pair}_{qc}")
                nc.tensor.matmul(qp[0:64, :], lhsT=wq_sb,
                                 rhs=xq0[:, qc * 512:(qc + 1) * 512],
                                 start=True, stop=True)
                nc.tensor.matmul(qp[64:128, :], lhsT=wq_sb,
                                 rhs=xq1[:, qc * 512:(qc + 1) * 512],
                                 start=True, stop=True)
                nc.scalar.copy(qp_sb[:, qc * 512:(qc + 1) * 512], qp)
            return go

        if fast_start:
            return ([half(kp_sb, 0, wk_sb, 0, "fk0"),
                     half(qp_sb, 2, wq_sb, 0, "fq0"),
                     half(kp_sb, 1, wk_sb, 1, "fk1"),
                     half(qp_sb, 3, wq_sb, 1, "fq1")]
                    + [kchunk(kc) for kc in range(1, 8)]
                    + [qchunk(qc) for qc in range(1, QW)])
        return ([kchunk(0), qchunk(0)] + [kchunk(kc) for kc in range(1, 8)]
                + [qchunk(qc) for qc in range(1, QW)])

    def attn(pair, qw, trickle=()):
        """One (head-pair, q-window): scores -> exp -> mask -> PV accumulate.

        Superseded by the global pipeline below; kept out of use."""
        raise NotImplementedError

    def finish_chunk(c):
        """Transpose concat chunk c into ct and run its out-projection."""
        def tr():
            ut = pstile(f"tr{c}")
            ut_bf = ut.bitcast(BF16)
            for pc in range(4):
                nc.tensor.transpose(ut_bf[:, pc * 128:(pc + 1) * 128],
                                    conc_sb[:, c * E + pc * 128:
                                            c * E + (pc + 1) * 128],
                                    ident_sb)
            ctv = bass.AP(tensor=ct_sb.tensor, offset=ct_sb.offset + c * 128,
                          ap=[ct_sb.ap[0], [QLEN, 4], [1, 128]])
            nc.vector.tensor_copy(ctv, ut_bf[:, 0:512])

        def op():
            o = pstile(f"op{c}")
            for pc in range(4):
                nc.tensor.matmul(o,
                                 lhsT=ct_sb[:, pc * QLEN + c * 128:
                                            pc * QLEN + (c + 1) * 128],
                                 rhs=wo_sb[pc],
                                 start=(pc == 0), stop=(pc == 3))
            osb = ospool.tile([128, E], F32, tag="osb", name=f"osb{c}")
            nc.vector.scalar_tensor_tensor(osb, o, 1.0, bo_sb,
                                           ALU.mult, ALU.add)
            nc.sync.dma_start(out[c * 128:(c + 1) * 128, :], osb)
        return [tr, op]

    # ---- global attention pipeline -------------------------------------
    # One continuous stream of 256 chunks (8 windows x 32 k-chunks); scores/
    # exp/mask run SKEW chunks ahead of the PV accumulation so neither PE nor
    # the elementwise engines ever wait on the in-flight chunk, including
    # across window boundaries.
    from collections import deque
    WINDOWS = [(p, w) for p in range(4) for w in range(QW)]
    SKEW = 8
    NG = len(WINDOWS) * KC
    Umap = {}
    pts = {}

    def emit_scores(g):
        w, kc = divmod(g, KC)
        pair, qw = WINDOWS[w]
        kp_sb = kproj_sb[pair]
        qp_sb = qproj_sb[pair]
        pt = ppool.tile([128, 1024], BF16, tag="pt", name=f"pt{w}_{kc}")
        pts[g] = pt
        pat = _EXP_PAT[g % len(_EXP_PAT)]
        for h2 in range(2):
            ps = psp.tile([128, 512], F32, tag="ps", name=f"ps{w}_{kc}_{h2}")
            nc.tensor.matmul(
                ps,
                lhsT=kp_sb[h2 * 64:h2 * 64 + 64, kc * 128:(kc + 1) * 128],
                rhs=qp_sb[h2 * 64:h2 * 64 + 64, qw * 512:(qw + 1) * 512],
                start=True, stop=True)
            pth = pt[:, h2 * 512:(h2 + 1) * 512]
            if pat[h2] == 'a':
                nc.scalar.activation(pth, ps, AF.Exp, bias=0.0, scale=0.125)
            else:
                e = nc.vector if pat[h2] == 'v' else nc.gpsimd
                e.tensor_scalar(pth.bitcast(I16), ps, EXPA, EXPB,
                                ALU.mult, ALU.add)
        # mask multiply (DVE, bf16 2x), broadcast across the head pair
        ms = mask_ap(kc, qw)
        mb = bass.AP(tensor=ms.tensor, offset=ms.offset,
                     ap=[ms.ap[0], [0, 2], [1, 512]])
        pv = pt.rearrange("p (h q) -> p h q", h=2)
        nc.vector.tensor_mul(pv, pv, mb)

    def emit_norms(w, inline_fins=False):
        """Normalize window w's context into concat staging. Runs on Pool
        (idle at window boundaries) as a single divide per (head, qsub):
        out = ctx_cols / denominator_col, broadcast from PSUM. For the last
        window the finish work is emitted eagerly per q-chunk."""
        pair, qw = WINDOWS[w]
        U = Umap[w]
        for h2 in range(2):
            h = 2 * pair + h2
            uin = bass.AP(tensor=U[h2].tensor, offset=U[h2].offset,
                          ap=[U[h2].ap[0], [65, 4], [1, 64]])
            den = bass.AP(tensor=U[h2].tensor, offset=U[h2].offset + 64,
                          ap=[U[h2].ap[0], [65, 4], [0, 64]])
            co = bass.AP(tensor=conc_sb.tensor,
                         offset=conc_sb.offset + (qw * 4) * E + h * 64,
                         ap=[conc_sb.ap[0], [E, 4], [1, 64]])
            nc.vector.scalar_tensor_tensor(co, uin, 1.0, den,
                                           ALU.mult, ALU.divide)
        if inline_fins:
            for qs in range(4):
                for work in finish_chunk(qw * 4 + qs):
                    work()

    def emit_pv(g):
        w, kc = divmod(g, KC)
        pair, qw = WINDOWS[w]
        if kc == 0:
            Umap[w] = [uacc.tile([128, 260], F32, tag=f"u{h2}",
                                 name=f"U{w}_{h2}")
                       for h2 in range(2)]
        U = Umap[w]
        pt = pts.pop(g)
        # One start=True per U bank marks the whole bank pending-zero; each
        # group's first write then zero-fills its own bytes.
        for h2 in range(2):
            h = 2 * pair + h2
            for qs in range(4):
                nc.tensor.matmul(
                    U[h2][:, qs * 65:qs * 65 + 65],
                    lhsT=pt[:, h2 * 512 + qs * 128:h2 * 512 + (qs + 1) * 128],
                    rhs=valp_ap(kc, h),
                    start=(kc == 0 and qs == 0), stop=(kc == KC - 1),
                    skip_group_check=True)
        if kc == KC - 1:
            emit_norms(w, inline_fins=(w == len(WINDOWS) - 1))

    # startup: xk0/xq0 first, then the first projection halves so their ACT
    # copies queue ahead of the bulk DMA configs, then everything else
    proj_load(0, only_first=True)
    fs = proj_chunks(0, fast_start=True)
    fs[0]()  # fk0 (k cols 0:512, both heads)
    fs[1]()  # fq0 (q window 0)
    proj_load(0, only_second=True)
    load_masks()
    load_valp()
    proj_load(1)
    load_late_consts()
    aux = deque(fs[2:])
    aux.extend(proj_chunks(1))
    # PV slot schedule: PV(kc) trails its scores by SKEW slots; the last 8
    # PVs of each window are compressed 2-per-slot so the window's normalize
    # lands early in the next window, ahead of that window's DVE mask queue.
    from collections import defaultdict
    pv_sched = defaultdict(list)
    for g in range(NG):
        w, kc = divmod(g, KC)
        s = g + SKEW if kc < KC - 8 else w * KC + KC + (kc - (KC - 8)) // 2
        pv_sched[s].append(g)
    last_slot = max(pv_sched)
    for g in range(max(NG, last_slot + 1)):
        if g == 44:
            proj_load(2)
        elif g == 56:
            aux.extend(proj_chunks(2))
        elif g == 108:
            proj_load(3)
        elif g == 120:
            aux.extend(proj_chunks(3))
        if g < NG:
            emit_scores(g)
        for gpv in pv_sched.get(g, ()):
            emit_pv(gpv)
            wv, kcv = divmod(gpv, KC)
            if kcv == KC - 1 and WINDOWS[wv] == (3, 0):
                for c in range(0, 4):
                    aux.extend(finish_chunk(c))
        if aux:
            aux.popleft()()
            if aux and g >= NG - 8:
                aux.popleft()()
    while aux:
        aux.popleft()()

    ctx.close()


def _prep_inputs(key, query, value, mask, Wq, Wk, Wv, Wo, bo):
    bf16 = ml_dtypes.bfloat16
    key = np.asarray(key, np.float32)
    query = np.asarray(query, np.float32)
    value = np.asarray(value, np.float32)
    mask = np.asarray(mask)
    Wv = np.asarray(Wv, np.float32)
    Wo = np.asarray(Wo, np.float32)
    # fold the V projection into the output projection:
    # concat_h(ctxraw_h @ Wv^T) @ Wo^T == concat_raw @ (Wo @ blockdiag(Wv))^T
    Wof = np.empty_like(Wo)
    for h in range(H):
        Wof[:, h * DH:(h + 1) * DH] = Wo[:, h * DH:(h + 1) * DH] @ Wv
    common = {
        "wqT": np.ascontiguousarray(np.asarray(Wq, np.float32).T).astype(bf16),
        "wkT": np.ascontiguousarray(np.asarray(Wk, np.float32).T).astype(bf16),
        "woT": np.ascontiguousarray(Wof.T).astype(bf16),
        "bo_b": np.ascontiguousarray(
            np.broadcast_to(np.asarray(bo, np.float32), (128, E))),
        "ident": np.eye(128, dtype=np.float32).astype(bf16),
    }
    maskT = np.ascontiguousarray(
        (mask[0, 0] != 0).astype(np.float32).T.astype(bf16))  # [k, q]
    per_b = {}
    for b in range(B):
        vp = np.ones((S, H, 65), np.float32)
        vp[:, :, :64] = value[b].reshape(S, H, DH)
        per_b[b] = {
            "xkT": np.ascontiguousarray(key[b].T).astype(bf16),
            "valp": np.ascontiguousarray(vp.reshape(S, H * 65).astype(bf16)),
            "qT": query[b].T,
        }
    in_maps = []
    for c in range(N_CORES):
        b, qs = c // 4, (c % 4) * QLEN
        in_maps.append({
            "xqT": np.ascontiguousarray(
                per_b[b]["qT"][:, qs:qs + QLEN]).astype(bf16),
            "xkT": per_b[b]["xkT"],
            "valp": per_b[b]["valp"],
            "maskT": np.ascontiguousarray(maskT[:, qs:qs + QLEN]),
            **common,
        })
    return in_maps


def get_module():
    if "nc" not in _CACHE:
        _CACHE["nc"] = _build_module()
    return _CACHE["nc"]


def kernel(key, query, value, mask, Wq, Wk, Wv, Wo, bo, **_):
    nc = get_module()
    in_maps = _prep_inputs(key, query, value, mask, Wq, Wk, Wv, Wo, bo)
    res = bass_utils.run_bass_kernel_spmd(
        nc, in_maps, core_ids=list(range(N_CORES)))
    full = np.empty((B, S, E), np.float32)
    for c in range(N_CORES):
        b, qs = c // 4, (c % 4) * QLEN
        full[b, qs:qs + QLEN, :] = res.results[c]["out"]
    return full


# revision 42
# speedup vs baseline: 1.0941x; 1.0164x over previous
"""Trainium2 Bass kernel for 8-head MultiHeadAttention (B=2, S=4096, E=512).

Sharding: 8 cores = 2 batches x 4 query-row chunks of 1024. Each core computes
all 8 heads for its (batch, q-range). Structure:
  - QK^T scores built transposed ([k partitions, q free]) as in the baseline.
  - softmax exp split across three engines: ACT (exact table exp) plus DVE and
    GPSIMD using a single-instruction Schraudolph bit-trick (int16 write
    bitcast to bf16), all masked multiplicatively afterward on DVE.
  - The attention-value matmul uses pt blocks as the stationary operand so the
    output lands as ctx[q partitions, d free] with a ones-column denominator:
    full 128-partition output halves the PE row count vs the [d, q] layout.
  - Wv is folded into Wo on the host (Wo' = Wo @ blockdiag(Wv)) so no V
    projection runs on device; normalization is a per-partition reciprocal
    plus a free-dim broadcast multiply straight into concat staging.
  - concat [q, e] is flipped to [e, q] via PE transposes against a host
    identity, then the output projection streams q rows per 128-q chunk.
"""
import sys
for _p in ('/root/.axon_site/_ro/trn_rl_repo', '/opt/trn_rl_repo'):
    if _p not in sys.path:
        sys.path.append(_p)

import numpy as np
import ml_dtypes

import concourse.bass as bass
import concourse.tile as tile
from concourse import bacc, mybir
from concourse import bass_utils

F32 = mybir.dt.float32
BF16 = mybir.dt.bfloat16
I16 = mybir.dt.int16
AF = mybir.ActivationFunctionType
ALU = mybir.AluOpType

N_CORES = 8
B, S, E, H, DH = 2, 4096, 512, 8, 64
QLEN = S // 4          # 1024 q rows per core
KC = S // 128          # 32 k chunks
QW = QLEN // 512       # 2 q windows of 512

# Schraudolph exp-as-bf16-bits: int16(x*EXPA + EXPB) bitcast bf16 ~ exp(x/8)
LOG2E = 1.4426950408889634
EXPA = 128.0 * LOG2E / 8.0
EXPB = 128.0 * (127.0 - 0.05735) + 0.5  # +0.5 compensates trunc-toward-zero

# exp engine assignment per half-chunk (each chunk's two head-halves exp on
# different engines in parallel): 'a'=ACT exact (612ns), 'p'=Pool bit-trick
# (850ns), 'v'=DVE bit-trick (594ns). Period 16 chunks: 17a/13p/2v keeps
# every engine under the PE chunk cadence in aggregate.
_EXP_PAT = [('a', 'p'), ('p', 'a'), ('a', 'a'), ('p', 'a'),
            ('a', 'p'), ('v', 'a'), ('p', 'a'), ('a', 'p'),
            ('p', 'a'), ('a', 'p'), ('a', 'a'), ('p', 'a'),
            ('a', 'p'), ('v', 'a'), ('p', 'a'), ('a', 'p')]

_CACHE = {}


def _build_module():
    nc = bacc.Bacc("TRN2", target_bir_lowering=False, debug=False,
                   enable_asserts=True, num_devices=N_CORES)

    xqT = nc.dram_tensor("xqT", [E, QLEN], BF16, kind="ExternalInput").ap()
    xkT = nc.dram_tensor("xkT", [E, S], BF16, kind="ExternalInput").ap()
    valp = nc.dram_tensor("valp", [S, H * 65], BF16, kind="ExternalInput").ap()
    maskT = nc.dram_tensor("maskT", [S, QLEN], BF16, kind="ExternalInput").ap()
    ident = nc.dram_tensor("ident", [128, 128], BF16, kind="ExternalInput").ap()
    wqT = nc.dram_tensor("wqT", [DH, DH], BF16, kind="ExternalInput").ap()
    wkT = nc.dram_tensor("wkT", [DH, DH], BF16, kind="ExternalInput").ap()
    woT = nc.dram_tensor("woT", [E, E], BF16, kind="ExternalInput").ap()
    bo_b = nc.dram_tensor("bo_b", [128, E], F32, kind="ExternalInput").ap()
    out = nc.dram_tensor("out", [QLEN, E], F32, kind="ExternalOutput").ap()

    with tile.TileContext(nc) as tc:
        _emit(tc, nc, xqT, xkT, valp, maskT, ident, wqT, wkT, woT, bo_b, out)

    nc.compile()
    return nc


def _emit(tc, nc, xqT, xkT, valp, maskT, ident, wqT, wkT, woT, bo_b, out):
    from contextlib import ExitStack
    ctx = ExitStack()
    const = ctx.enter_context(tc.tile_pool(name="const", bufs=1))
    kpool = ctx.enter_context(tc.tile_pool(name="kproj", bufs=1))
    qpool = ctx.enter_context(tc.tile_pool(name="qproj", bufs=2))
    xkst = ctx.enter_context(tc.tile_pool(name="xkst", bufs=2))
    ppool = ctx.enter_context(tc.tile_pool(name="p", bufs=10))
    rcpool = ctx.enter_context(tc.tile_pool(name="rc", bufs=2))
    ospool = ctx.enter_context(tc.tile_pool(name="osb", bufs=2))
    psp = ctx.enter_context(tc.tile_pool(name="psp", bufs=6, space="PSUM"))
    uacc = ctx.enter_context(tc.tile_pool(name="uacc", bufs=1, space="PSUM"))

    def pstile(nm):
        """Scratch PSUM [128, 512] from the shared ps rotation (one bank)."""
        return psp.tile([128, 512], F32, tag="ps", name=nm)

    # ---- resident mask tiles, 4 k-chunks per tile (loaded once, batched
    # DMAs: HWDGE descriptor generation is ~630ns per dma_start, so window 0
    # can't afford one DMA per 128-row chunk) ----
    mask_res = [const.tile([128, 4 * QLEN], BF16, tag=f"mk{c}", name=f"mk{c}")
                for c in range(KC // 4)]

    def mask_ap(kc, qw):
        t = mask_res[kc // 4]
        return t[:, (kc % 4) * QLEN + qw * 512:(kc % 4) * QLEN + qw * 512 + 512]

    def load_masks():
        for c in range(KC // 4):
            dst = mask_res[c].rearrange("p (c q) -> p c q", c=4)
            src = bass.AP(tensor=maskT.tensor, offset=c * 512 * QLEN,
                          ap=[[QLEN, 128], [128 * QLEN, 4], [1, QLEN]])
            nc.sync.dma_start(dst, src)

    # ---- constants: wq/wk ride the sync queue ahead of xk0 (fk0 needs both);
    # wo/bo/ident desc-gen late on gpsimd (needed only in the final phase)
    wq_sb = const.tile([DH, DH], BF16, tag="wq")
    nc.sync.dma_start(wq_sb, wqT)
    wk_sb = const.tile([DH, DH], BF16, tag="wk")
    nc.sync.dma_start(wk_sb, wkT)
    ident_sb = const.tile([128, 128], BF16, tag="ident")
    wo_sb = []
    for pc in range(4):
        wo_sb.append(const.tile([128, E], BF16, tag=f"wo{pc}", name=f"wo{pc}"))
    bo_sb = const.tile([128, E], F32, tag="bo")

    def load_late_consts():
        nc.gpsimd.dma_start(ident_sb, ident)
        for pc in range(4):
            nc.gpsimd.dma_start(wo_sb[pc], woT[pc * 128:(pc + 1) * 128, :])
        nc.gpsimd.dma_start(bo_sb, bo_b)

    # valp, 4 k-chunks per tile, batched DMAs on the ACT HWDGE queue
    VW = H * 65
    valp_t = [const.tile([128, 4 * VW], BF16, tag=f"vp{c}", name=f"vp{c}")
              for c in range(KC // 4)]

    def load_valp():
        for c in range(KC // 4):
            dst = valp_t[c].rearrange("p (c v) -> p c v", c=4)
            src = bass.AP(tensor=valp.tensor, offset=c * 512 * VW,
                          ap=[[VW, 128], [128 * VW, 4], [1, VW]])
            nc.scalar.dma_start(dst, src)

    def valp_ap(kc, h):
        t = valp_t[kc // 4]
        return t[:, (kc % 4) * VW + h * 65:(kc % 4) * VW + h * 65 + 65]

    # concat staging [q 128, E] bf16, all 8 q-chunks in one tile so the
    # normalize divide can hit all 4 qsubs of a window in one instruction
    conc_sb = const.tile([128, 8 * E], BF16, tag="cq")
    # transposed concat [e, q] for the out-projection lhsT: 4 pc-blocks x QLEN
    ct_sb = const.tile([128, 4 * QLEN], BF16, tag="ct")

    kproj_sb = [None] * 4
    qproj_sb = [None] * 4
    xs = {}

    def proj_load(pair, only_first=False, only_second=False):
        # all pairs on the sync/scalar HWDGE queues; gpsimd desc-gen would
        # queue behind Pool's exp work and arrive tens of us late
        qk = nc.sync
        qq = nc.scalar
        if not only_second:
            kproj_sb[pair] = kpool.tile([128, S], BF16, tag=f"kp{pair}",
                                        name=f"kp{pair}")
            qproj_sb[pair] = qpool.tile([128, QLEN], BF16, tag="qp",
                                        name=f"qp{pair}")
            xk0 = xkst.tile([DH, S], BF16, tag="xk", name=f"xk0_{pair}")
            qk.dma_start(xk0, xkT[(2 * pair) * DH:(2 * pair + 1) * DH, :])
            xq0 = xkst.tile([DH, QLEN], BF16, tag="xq", name=f"xq0_{pair}")
            qq.dma_start(xq0, xqT[(2 * pair) * DH:(2 * pair + 1) * DH, :])
            xs[pair] = (xk0, None, xq0, None)
            if only_first:
                return
        xk0, _, xq0, _ = xs[pair]
        xk1 = xkst.tile([DH, S], BF16, tag="xk", name=f"xk1_{pair}")
        qk.dma_start(xk1, xkT[(2 * pair + 1) * DH:(2 * pair + 2) * DH, :])
        xq1 = xkst.tile([DH, QLEN], BF16, tag="xq", name=f"xq1_{pair}")
        qq.dma_start(xq1, xqT[(2 * pair + 1) * DH:(2 * pair + 2) * DH, :])
        xs[pair] = (xk0, xk1, xq0, xq1)

    def proj_chunks(pair, fast_start=False):
        """Closures: 8 kproj chunks + 2 qproj chunks. Copies run on ACT.
        xs[pair] is read lazily so loads may be staged in two steps."""
        kp_sb = kproj_sb[pair]
        qp_sb = qproj_sb[pair]

        def pcopy(i, dst, src):
            # spread projection copies across ACT and Pool
            if i % 2 == 0:
                nc.scalar.copy(dst, src)
            else:
                nc.gpsimd.tensor_copy(dst, src)

        def half(dst, src_i, w, h2, nm):
            def go():
                t = pstile(nm)
                lo, hi = h2 * 64, (h2 + 1) * 64
                nc.tensor.matmul(t[lo:hi, :], lhsT=w,
                                 rhs=xs[pair][src_i][:, 0:512],
                                 start=True, stop=True)
                nc.scalar.copy(dst[lo:hi, 0:512], t[lo:hi, :])
            return go

        def kchunk(kc):
            def go():
                xk0, xk1 = xs[pair][0], xs[pair][1]
                kp = pstile(f"kpp{pair}_{kc}")
                nc.tensor.matmul(kp[0:64, :], lhsT=wk_sb,
                                 rhs=xk0[:, kc * 512:(kc + 1) * 512],
                                 start=True, stop=True)
                nc.tensor.matmul(kp[64:128, :], lhsT=wk_sb,
                                 rhs=xk1[:, kc * 512:(kc + 1) * 512],
                                 start=True, stop=True)
                pcopy(kc, kp_sb[:, kc * 512:(kc + 1) * 512], kp)
            return go

        def qchunk(qc):
            def go():
                xq0, xq1 = xs[pair][2], xs[pair][3]
                qp = pstile(f"qpp{pair}_{qc}")
                nc.tensor.matmul(qp[0:64, :], lhsT=wq_sb,
                                 rhs=xq0[:, qc * 512:(qc + 1) * 512],
                                 start=True, stop=True)
                nc.tensor.matmul(qp[64:128, :], lhsT=wq_sb,
                                 rhs=xq1[:, qc * 512:(qc + 1) * 512],
                                 start=True, stop=True)
                pcopy(qc + 1, qp_sb[:, qc * 512:(qc + 1) * 512], qp)
            return go

        if fast_start:
            return ([half(kp_sb, 0, wk_sb, 0, "fk0"),
                     half(qp_sb, 2, wq_sb, 0, "fq0"),
                     half(kp_sb, 1, wk_sb, 1, "fk1"),
                     half(qp_sb, 3, wq_sb, 1, "fq1")]
                    + [kchunk(kc) for kc in range(1, 8)]
                    + [qchunk(qc) for qc in range(1, QW)])
        return ([kchunk(0), qchunk(0)] + [kchunk(kc) for kc in range(1, 8)]
                + [qchunk(qc) for qc in range(1, QW)])

    def attn(pair, qw, trickle=()):
        """One (head-pair, q-window): scores -> exp -> mask -> PV accumulate.

        Superseded by the global pipeline below; kept out of use."""
        raise NotImplementedError

    def finish_chunk(c):
        """Transpose concat chunk c into ct and run its out-projection."""
        def tr():
            ut = pstile(f"tr{c}")
            ut_bf = ut.bitcast(BF16)
            for pc in range(4):
                nc.tensor.transpose(ut_bf[:, pc * 128:(pc + 1) * 128],
                                    conc_sb[:, c * E + pc * 128:
                                            c * E + (pc + 1) * 128],
                                    ident_sb)
            ctv = bass.AP(tensor=ct_sb.tensor, offset=ct_sb.offset + c * 128,
                          ap=[ct_sb.ap[0], [QLEN, 4], [1, 128]])
            nc.vector.tensor_copy(ctv, ut_bf[:, 0:512])

        def op():
            o = pstile(f"op{c}")
            for pc in range(4):
                nc.tensor.matmul(o,
                                 lhsT=ct_sb[:, pc * QLEN + c * 128:
                                            pc * QLEN + (c + 1) * 128],
                                 rhs=wo_sb[pc],
                                 start=(pc == 0), stop=(pc == 3))
            osb = ospool.tile([128, E], F32, tag="osb", name=f"osb{c}")
            nc.vector.scalar_tensor_tensor(osb, o, 1.0, bo_sb,
                                           ALU.mult, ALU.add)
            nc.sync.dma_start(out[c * 128:(c + 1) * 128, :], osb)
        return [tr, op]

    # ---- global attention pipeline -------------------------------------
    # One continuous stream of 256 chunks (8 windows x 32 k-chunks); scores/
    # exp/mask run SKEW chunks ahead of the PV accumulation so neither PE nor
    # the elementwise engines ever wait on the in-flight chunk, including
    # across window boundaries.
    from collections import deque
    WINDOWS = [(p, w) for p in range(4) for w in range(QW)]
    SKEW = 8
    NG = len(WINDOWS) * KC
    Umap = {}
    pts = {}

    def emit_scores(g):
        w, kc = divmod(g, KC)
        pair, qw = WINDOWS[w]
        kp_sb = kproj_sb[pair]
        qp_sb = qproj_sb[pair]
        pt = ppool.tile([128, 1024], BF16, tag="pt", name=f"pt{w}_{kc}")
        pts[g] = pt
        pat = _EXP_PAT[g % len(_EXP_PAT)]
        for h2 in range(2):
            ps = psp.tile([128, 512], F32, tag="ps", name=f"ps{w}_{kc}_{h2}")
            nc.tensor.matmul(
                ps,
                lhsT=kp_sb[h2 * 64:h2 * 64 + 64, kc * 128:(kc + 1) * 128],
                rhs=qp_sb[h2 * 64:h2 * 64 + 64, qw * 512:(qw + 1) * 512],
                start=True, stop=True)
            pth = pt[:, h2 * 512:(h2 + 1) * 512]
            if pat[h2] == 'a':
                nc.scalar.activation(pth, ps, AF.Exp, bias=0.0, scale=0.125)
            else:
                e = nc.vector if pat[h2] == 'v' else nc.gpsimd
                e.tensor_scalar(pth.bitcast(I16), ps, EXPA, EXPB,
                                ALU.mult, ALU.add)
        # mask multiply (DVE, bf16 2x), broadcast across the head pair
        ms = mask_ap(kc, qw)
        mb = bass.AP(tensor=ms.tensor, offset=ms.offset,
                     ap=[ms.ap[0], [0, 2], [1, 512]])
        pv = pt.rearrange("p (h q) -> p h q", h=2)
        nc.vector.tensor_mul(pv, pv, mb)

    def emit_norms(w, inline_fins=False):
        """Normalize window w's context into concat staging. Runs on Pool
        (idle at window boundaries) as a single divide per (head, qsub):
        out = ctx_cols / denominator_col, broadcast from PSUM. For the last
        window the finish work is emitted eagerly per q-chunk."""
        pair, qw = WINDOWS[w]
        U = Umap[w]
        for h2 in range(2):
            h = 2 * pair + h2
            uin = bass.AP(tensor=U[h2].tensor, offset=U[h2].offset,
                          ap=[U[h2].ap[0], [65, 4], [1, 64]])
            den = bass.AP(tensor=U[h2].tensor, offset=U[h2].offset + 64,
                          ap=[U[h2].ap[0], [65, 4], [0, 64]])
            co = bass.AP(tensor=conc_sb.tensor,
                         offset=conc_sb.offset + (qw * 4) * E + h * 64,
                         ap=[conc_sb.ap[0], [E, 4], [1, 64]])
            nc.vector.scalar_tensor_tensor(co, uin, 1.0, den,
                                           ALU.mult, ALU.divide)
        if inline_fins:
            for qs in range(4):
                for work in finish_chunk(qw * 4 + qs):
                    work()

    def emit_pv(g):
        w, kc = divmod(g, KC)
        pair, qw = WINDOWS[w]
        if kc == 0:
            Umap[w] = [uacc.tile([128, 260], F32, tag=f"u{h2}",
                                 name=f"U{w}_{h2}")
                       for h2 in range(2)]
        U = Umap[w]
        pt = pts.pop(g)
        # One start=True per U bank marks the whole bank pending-zero; each
        # group's first write then zero-fills its own bytes.
        for h2 in range(2):
            h = 2 * pair + h2
            for qs in range(4):
                nc.tensor.matmul(
                    U[h2][:, qs * 65:qs * 65 + 65],
                    lhsT=pt[:, h2 * 512 + qs * 128:h2 * 512 + (qs + 1) * 128],
                    rhs=valp_ap(kc, h),
                    start=(kc == 0 and qs == 0), stop=(kc == KC - 1),
                    skip_group_check=True)
        if kc == KC - 1:
            emit_norms(w, inline_fins=(w == len(WINDOWS) - 1))

    # startup: xk0/xq0 first, then the first projection halves so their ACT
    # copies queue ahead of the bulk DMA configs, then everything else
    proj_load(0, only_first=True)
    fs = proj_chunks(0, fast_start=True)
    fs[0]()  # fk0 (k cols 0:512, both heads)
    fs[1]()  # fq0 (q window 0)
    proj_load(0, only_second=True)
    load_masks()
    load_valp()
    proj_load(1)
    aux = deque(fs[2:])
    # PV slot schedule: PV(kc) trails its scores by SKEW slots; the last 8
    # PVs of each window are compressed 2-per-slot so the window's normalize
    # lands early in the next window, ahead of that window's DVE mask queue.
    from collections import defaultdict
    pv_sched = defaultdict(list)
    for g in range(NG):
        w, kc = divmod(g, KC)
        s = g + SKEW if kc < KC - 8 else w * KC + KC + (kc - (KC - 8)) // 2
        pv_sched[s].append(g)
    last_slot = max(pv_sched)
    for g in range(max(NG, last_slot + 1)):
        if g == 14:
            aux.extend(proj_chunks(1))
        elif g == 30:
            load_late_consts()
        elif g == 44:
            proj_load(2)
        elif g == 56:
            aux.extend(proj_chunks(2))
        elif g == 108:
            proj_load(3)
        elif g == 120:
            aux.extend(proj_chunks(3))
        if g < NG:
            emit_scores(g)
        for gpv in pv_sched.get(g, ()):
            emit_pv(gpv)
            wv, kcv = divmod(gpv, KC)
            if kcv == KC - 1 and WINDOWS[wv] == (3, 0):
                for c in range(0, 4):
                    aux.extend(finish_chunk(c))
        if aux:
            aux.popleft()()
            if aux and g >= NG - 8:
                aux.popleft()()
    while aux:
        aux.popleft()()

    ctx.close()


def _prep_inputs(key, query, value, mask, Wq, Wk, Wv, Wo, bo):
    bf16 = ml_dtypes.bfloat16
    key = np.asarray(key, np.float32)
    query = np.asarray(query, np.float32)
    value = np.asarray(value, np.float32)
    mask = np.asarray(mask)
    Wv = np.asarray(Wv, np.float32)
    Wo = np.asarray(Wo, np.float32)
    # fold the V projection into the output projection:
    # concat_h(ctxraw_h @ Wv^T) @ Wo^T == concat_raw @ (Wo @ blockdiag(Wv))^T
    Wof = np.empty_like(Wo)
    for h in range(H):
        Wof[:, h * DH:(h + 1) * DH] = Wo[:, h * DH:(h + 1) * DH] @ Wv
    common = {
        "wqT": np.ascontiguousarray(np.asarray(Wq, np.float32).T).astype(bf16),
        "wkT": np.ascontiguousarray(np.asarray(Wk, np.float32).T).astype(bf16),
        "woT": np.ascontiguousarray(Wof.T).astype(bf16),
        "bo_b": np.ascontiguousarray(
            np.broadcast_to(np.asarray(bo, np.float32), (128, E))),
        "ident": np.eye(128, dtype=np.float32).astype(bf16),
    }
    maskT = np.ascontiguousarray(
        (mask[0, 0] != 0).astype(np.float32).T.astype(bf16))  # [k, q]
    per_b = {}
    for b in range(B):
        vp = np.ones((S, H, 65), np.float32)
        vp[:, :, :64] = value[b].reshape(S, H, DH)
        per_b[b] = {
            "xkT": np.ascontiguousarray(key[b].T).astype(bf16),
            "valp": np.ascontiguousarray(vp.reshape(S, H * 65).astype(bf16)),
            "qT": query[b].T,
        }
    in_maps = []
    for c in range(N_CORES):
        b, qs = c // 4, (c % 4) * QLEN
        in_maps.append({
            "xqT": np.ascontiguousarray(
                per_b[b]["qT"][:, qs:qs + QLEN]).astype(bf16),
            "xkT": per_b[b]["xkT"],
            "valp": per_b[b]["valp"],
            "maskT": np.ascontiguousarray(maskT[:, qs:qs + QLEN]),
            **common,
        })
    return in_maps


def get_module():
    if "nc" not in _CACHE:
        _CACHE["nc"] = _build_module()
    return _CACHE["nc"]


def kernel(key, query, value, mask, Wq, Wk, Wv, Wo, bo, **_):
    nc = get_module()
    in_maps = _prep_inputs(key, query, value, mask, Wq, Wk, Wv, Wo, bo)
    res = bass_utils.run_bass_kernel_spmd(
        nc, in_maps, core_ids=list(range(N_CORES)))
    full = np.empty((B, S, E), np.float32)
    for c in range(N_CORES):
        b, qs = c // 4, (c % 4) * QLEN
        full[b, qs:qs + QLEN, :] = res.results[c]["out"]
    return full


# revision 50
# speedup vs baseline: 1.1666x; 1.0663x over previous
"""Trainium2 Bass kernel for 8-head MultiHeadAttention (B=2, S=4096, E=512).

Sharding: 8 cores = 2 batches x 4 query-row chunks of 1024. Each core computes
all 8 heads for its (batch, q-range). Structure:
  - QK^T scores built transposed ([k partitions, q free]) as in the baseline.
  - softmax exp split across three engines: ACT (exact table exp) plus DVE and
    GPSIMD using a single-instruction Schraudolph bit-trick (int16 write
    bitcast to bf16), all masked multiplicatively afterward on DVE.
  - The attention-value matmul uses pt blocks as the stationary operand so the
    output lands as ctx[q partitions, d free] with a ones-column denominator:
    full 128-partition output halves the PE row count vs the [d, q] layout.
  - Wv is folded into Wo on the host (Wo' = Wo @ blockdiag(Wv)) so no V
    projection runs on device; normalization is a per-partition reciprocal
    plus a free-dim broadcast multiply straight into concat staging.
  - concat [q, e] is flipped to [e, q] via PE transposes against a host
    identity, then the output projection streams q rows per 128-q chunk.
"""
import sys
for _p in ('/root/.axon_site/_ro/trn_rl_repo', '/opt/trn_rl_repo'):
    if _p not in sys.path:
        sys.path.append(_p)

import numpy as np
import ml_dtypes

import concourse.bass as bass
import concourse.tile as tile
from concourse import bacc, mybir
from concourse import bass_utils

F32 = mybir.dt.float32
BF16 = mybir.dt.bfloat16
I16 = mybir.dt.int16
AF = mybir.ActivationFunctionType
ALU = mybir.AluOpType

N_CORES = 8
B, S, E, H, DH = 2, 4096, 512, 8, 64
QLEN = S // 4          # 1024 q rows per core
KC = S // 128          # 32 k chunks
QW = QLEN // 512       # 2 q windows of 512

# Schraudolph exp-as-bf16-bits: int16(x*EXPA + EXPB) bitcast bf16 ~ exp(x/8)
LOG2E = 1.4426950408889634
EXPA = 128.0 * LOG2E / 8.0
EXPB = 128.0 * (127.0 - 0.05735) + 0.5  # +0.5 compensates trunc-toward-zero

# exp engine assignment per half-chunk (each chunk's two head-halves exp on
# different engines in parallel): 'a'=ACT exact (612ns), 'p'=Pool bit-trick
# (850ns), 'v'=DVE bit-trick (594ns). Period 16 chunks: 17a/13p/2v keeps
# every engine under the PE chunk cadence in aggregate.
_EXP_PAT = [('a', 'p'), ('p', 'a'), ('a', 'a'), ('p', 'a'),
            ('a', 'p'), ('v', 'a'), ('p', 'a'), ('a', 'p'),
            ('p', 'a'), ('a', 'p'), ('a', 'a'), ('p', 'a'),
            ('a', 'p'), ('v', 'a'), ('p', 'a'), ('a', 'p')]

_CACHE = {}


def _build_module():
    nc = bacc.Bacc("TRN2", target_bir_lowering=False, debug=False,
                   enable_asserts=True, num_devices=N_CORES)

    xqT = nc.dram_tensor("xqT", [E, QLEN], BF16, kind="ExternalInput").ap()
    xkT = nc.dram_tensor("xkT", [E, S], BF16, kind="ExternalInput").ap()
    valp = nc.dram_tensor("valp", [S, H * 65], BF16, kind="ExternalInput").ap()
    maskT = nc.dram_tensor("maskT", [S, QLEN], BF16, kind="ExternalInput").ap()
    ident = nc.dram_tensor("ident", [128, 128], BF16, kind="ExternalInput").ap()
    wqT = nc.dram_tensor("wqT", [DH, DH], BF16, kind="ExternalInput").ap()
    wkT = nc.dram_tensor("wkT", [DH, DH], BF16, kind="ExternalInput").ap()
    woT = nc.dram_tensor("woT", [E, E], BF16, kind="ExternalInput").ap()
    bo_b = nc.dram_tensor("bo_b", [128, E], F32, kind="ExternalInput").ap()
    out = nc.dram_tensor("out", [QLEN, E], F32, kind="ExternalOutput").ap()

    with tile.TileContext(nc) as tc:
        _emit(tc, nc, xqT, xkT, valp, maskT, ident, wqT, wkT, woT, bo_b, out)

    nc.compile()
    return nc


def _emit(tc, nc, xqT, xkT, valp, maskT, ident, wqT, wkT, woT, bo_b, out):
    from contextlib import ExitStack
    ctx = ExitStack()
    const = ctx.enter_context(tc.tile_pool(name="const", bufs=1))
    kpool = ctx.enter_context(tc.tile_pool(name="kproj", bufs=1))
    qpool = ctx.enter_context(tc.tile_pool(name="qproj", bufs=2))
    xkst = ctx.enter_context(tc.tile_pool(name="xkst", bufs=2))
    ppool = ctx.enter_context(tc.tile_pool(name="p", bufs=10))
    rcpool = ctx.enter_context(tc.tile_pool(name="rc", bufs=2))
    ospool = ctx.enter_context(tc.tile_pool(name="osb", bufs=2))
    psp = ctx.enter_context(tc.tile_pool(name="psp", bufs=6, space="PSUM"))
    uacc = ctx.enter_context(tc.tile_pool(name="uacc", bufs=1, space="PSUM"))

    def pstile(nm):
        """Scratch PSUM [128, 512] from the shared ps rotation (one bank)."""
        return psp.tile([128, 512], F32, tag="ps", name=nm)

    # ---- resident mask tiles, 4 k-chunks per tile (loaded once, batched
    # DMAs: HWDGE descriptor generation is ~630ns per dma_start, so window 0
    # can't afford one DMA per 128-row chunk) ----
    mask_res = [const.tile([128, 4 * QLEN], BF16, tag=f"mk{c}", name=f"mk{c}")
                for c in range(KC // 4)]

    def mask_ap(kc, qw):
        t = mask_res[kc // 4]
        return t[:, (kc % 4) * QLEN + qw * 512:(kc % 4) * QLEN + qw * 512 + 512]

    def load_masks(qw, cs):
        """Load the qw-half of mask tiles cs (window 0 only needs qw=0)."""
        for c in cs:
            t = mask_res[c]
            dst = bass.AP(tensor=t.tensor, offset=t.offset + qw * 512,
                          ap=[t.ap[0], [QLEN, 4], [1, 512]])
            src = bass.AP(tensor=maskT.tensor,
                          offset=c * 512 * QLEN + qw * 512,
                          ap=[[QLEN, 128], [128 * QLEN, 4], [1, 512]])
            nc.sync.dma_start(dst, src)

    # ---- constants: wq/wk ride the sync queue ahead of xk0 (fk0 needs both);
    # wo/bo/ident desc-gen late on gpsimd (needed only in the final phase)
    wq_sb = const.tile([DH, DH], BF16, tag="wq")
    nc.sync.dma_start(wq_sb, wqT)
    wk_sb = const.tile([DH, DH], BF16, tag="wk")
    nc.sync.dma_start(wk_sb, wkT)
    ident_sb = const.tile([128, 128], BF16, tag="ident")
    wo_sb = []
    for pc in range(4):
        wo_sb.append(const.tile([128, E], BF16, tag=f"wo{pc}", name=f"wo{pc}"))
    bo_sb = const.tile([128, E], F32, tag="bo")

    def load_late_consts():
        nc.gpsimd.dma_start(ident_sb, ident)
        for pc in range(4):
            nc.gpsimd.dma_start(wo_sb[pc], woT[pc * 128:(pc + 1) * 128, :])
        nc.gpsimd.dma_start(bo_sb, bo_b)

    # valp, 4 k-chunks per tile, batched DMAs on the ACT HWDGE queue
    VW = H * 65
    valp_t = [const.tile([128, 4 * VW], BF16, tag=f"vp{c}", name=f"vp{c}")
              for c in range(KC // 4)]

    def load_valp(cs):
        for c in cs:
            dst = valp_t[c].rearrange("p (c v) -> p c v", c=4)
            src = bass.AP(tensor=valp.tensor, offset=c * 512 * VW,
                          ap=[[VW, 128], [128 * VW, 4], [1, VW]])
            nc.scalar.dma_start(dst, src)

    def valp_ap(kc, h):
        t = valp_t[kc // 4]
        return t[:, (kc % 4) * VW + h * 65:(kc % 4) * VW + h * 65 + 65]

    # concat staging [q 128, E] bf16, all 8 q-chunks in one tile so the
    # normalize divide can hit all 4 qsubs of a window in one instruction
    conc_sb = const.tile([128, 8 * E], BF16, tag="cq")
    # transposed concat [e, q] for the out-projection lhsT: 4 pc-blocks x QLEN
    ct_sb = const.tile([128, 4 * QLEN], BF16, tag="ct")

    kproj_sb = [None] * 4
    qproj_sb = [None] * 4
    xs = {}

    def proj_load(pair):
        # full-128-partition tiles (both heads stacked): the DMA pipe charges
        # per-partition-line bytes, so 64-row loads would waste half of it.
        # sync/scalar HWDGE queues; gpsimd desc-gen would queue behind Pool.
        kproj_sb[pair] = kpool.tile([128, S], BF16, tag=f"kp{pair}",
                                    name=f"kp{pair}")
        qproj_sb[pair] = qpool.tile([128, QLEN], BF16, tag="qp",
                                    name=f"qp{pair}")
        xk = xkst.tile([128, S], BF16, tag="xk", name=f"xk_{pair}")
        nc.sync.dma_start(xk, xkT[(2 * pair) * DH:(2 * pair + 2) * DH, :])
        xq = xkst.tile([128, QLEN], BF16, tag="xq", name=f"xq_{pair}")
        nc.scalar.dma_start(xq, xqT[(2 * pair) * DH:(2 * pair + 2) * DH, :])
        xs[pair] = (xk, xq)

    def proj_chunks(pair, fast_start=False):
        """Closures: 8 kproj chunks + 2 qproj chunks. Copies run on ACT.
        xs[pair] is read lazily so loads may be staged in two steps."""
        kp_sb = kproj_sb[pair]
        qp_sb = qproj_sb[pair]

        def pcopy(i, dst, src):
            # spread projection copies across ACT and Pool
            if i % 2 == 0:
                nc.scalar.copy(dst, src)
            else:
                nc.gpsimd.tensor_copy(dst, src)

        def kchunk(kc):
            def go():
                xk = xs[pair][0]
                kp = pstile(f"kpp{pair}_{kc}")
                nc.tensor.matmul(kp[0:64, :], lhsT=wk_sb,
                                 rhs=xk[0:64, kc * 512:(kc + 1) * 512],
                                 start=True, stop=True)
                nc.tensor.matmul(kp[64:128, :], lhsT=wk_sb,
                                 rhs=xk[64:128, kc * 512:(kc + 1) * 512],
                                 start=True, stop=True,
                                 tile_position=(64, 64))
                pcopy(kc, kp_sb[:, kc * 512:(kc + 1) * 512], kp)
            return go

        def qchunk(qc):
            def go():
                xq = xs[pair][1]
                qp = pstile(f"qpp{pair}_{qc}")
                nc.tensor.matmul(qp[0:64, :], lhsT=wq_sb,
                                 rhs=xq[0:64, qc * 512:(qc + 1) * 512],
                                 start=True, stop=True)
                nc.tensor.matmul(qp[64:128, :], lhsT=wq_sb,
                                 rhs=xq[64:128, qc * 512:(qc + 1) * 512],
                                 start=True, stop=True,
                                 tile_position=(64, 64))
                pcopy(qc + 1, qp_sb[:, qc * 512:(qc + 1) * 512], qp)
            return go

        return ([kchunk(0), qchunk(0)] + [kchunk(kc) for kc in range(1, 8)]
                + [qchunk(qc) for qc in range(1, QW)])

    def attn(pair, qw, trickle=()):
        """One (head-pair, q-window): scores -> exp -> mask -> PV accumulate.

        Superseded by the global pipeline below; kept out of use."""
        raise NotImplementedError

    def finish_chunk(c):
        """Transpose concat chunk c into ct and run its out-projection."""
        def tr():
            ut = pstile(f"tr{c}")
            ut_bf = ut.bitcast(BF16)
            for pc in range(4):
                nc.tensor.transpose(ut_bf[:, pc * 128:(pc + 1) * 128],
                                    conc_sb[:, c * E + pc * 128:
                                            c * E + (pc + 1) * 128],
                                    ident_sb)
            ctv = bass.AP(tensor=ct_sb.tensor, offset=ct_sb.offset + c * 128,
                          ap=[ct_sb.ap[0], [QLEN, 4], [1, 128]])
            nc.vector.tensor_copy(ctv, ut_bf[:, 0:512])

        def op():
            o = pstile(f"op{c}")
            for pc in range(4):
                nc.tensor.matmul(o,
                                 lhsT=ct_sb[:, pc * QLEN + c * 128:
                                            pc * QLEN + (c + 1) * 128],
                                 rhs=wo_sb[pc],
                                 start=(pc == 0), stop=(pc == 3))
            osb = ospool.tile([128, E], F32, tag="osb", name=f"osb{c}")
            nc.vector.scalar_tensor_tensor(osb, o, 1.0, bo_sb,
                                           ALU.mult, ALU.add)
            nc.sync.dma_start(out[c * 128:(c + 1) * 128, :], osb)
        return [tr, op]

    # ---- global attention pipeline -------------------------------------
    # One continuous stream of 256 chunks (8 windows x 32 k-chunks); scores/
    # exp/mask run SKEW chunks ahead of the PV accumulation so neither PE nor
    # the elementwise engines ever wait on the in-flight chunk, including
    # across window boundaries.
    from collections import deque
    WINDOWS = [(p, w) for p in range(4) for w in range(QW)]
    SKEW = 8
    NG = len(WINDOWS) * KC
    Umap = {}
    pts = {}

    def emit_scores(g):
        w, kc = divmod(g, KC)
        pair, qw = WINDOWS[w]
        kp_sb = kproj_sb[pair]
        qp_sb = qproj_sb[pair]
        pt = ppool.tile([128, 1024], BF16, tag="pt", name=f"pt{w}_{kc}")
        pts[g] = pt
        pat = _EXP_PAT[g % len(_EXP_PAT)]
        for h2 in range(2):
            ps = psp.tile([128, 512], F32, tag="ps", name=f"ps{w}_{kc}_{h2}")
            nc.tensor.matmul(
                ps,
                lhsT=kp_sb[h2 * 64:h2 * 64 + 64, kc * 128:(kc + 1) * 128],
                rhs=qp_sb[h2 * 64:h2 * 64 + 64, qw * 512:(qw + 1) * 512],
                start=True, stop=True)
            pth = pt[:, h2 * 512:(h2 + 1) * 512]
            if pat[h2] == 'a':
                nc.scalar.activation(pth, ps, AF.Exp, bias=0.0, scale=0.125)
            else:
                e = nc.vector if pat[h2] == 'v' else nc.gpsimd
                e.tensor_scalar(pth.bitcast(I16), ps, EXPA, EXPB,
                                ALU.mult, ALU.add)
        # mask multiply (DVE, bf16 2x), broadcast across the head pair
        ms = mask_ap(kc, qw)
        mb = bass.AP(tensor=ms.tensor, offset=ms.offset,
                     ap=[ms.ap[0], [0, 2], [1, 512]])
        pv = pt.rearrange("p (h q) -> p h q", h=2)
        nc.vector.tensor_mul(pv, pv, mb)

    def emit_norms(w, inline_fins=False):
        """Normalize window w's context into concat staging. Runs on Pool
        (idle at window boundaries) as a single divide per (head, qsub):
        out = ctx_cols / denominator_col, broadcast from PSUM. For the last
        window the finish work is emitted eagerly per q-chunk."""
        pair, qw = WINDOWS[w]
        U = Umap[w]
        for h2 in range(2):
            h = 2 * pair + h2
            uin = bass.AP(tensor=U[h2].tensor, offset=U[h2].offset,
                          ap=[U[h2].ap[0], [65, 4], [1, 64]])
            den = bass.AP(tensor=U[h2].tensor, offset=U[h2].offset + 64,
                          ap=[U[h2].ap[0], [65, 4], [0, 64]])
            co = bass.AP(tensor=conc_sb.tensor,
                         offset=conc_sb.offset + (qw * 4) * E + h * 64,
                         ap=[conc_sb.ap[0], [E, 4], [1, 64]])
            nc.vector.scalar_tensor_tensor(co, uin, 1.0, den,
                                           ALU.mult, ALU.divide)
        if inline_fins:
            for qs in range(4):
                for work in finish_chunk(qw * 4 + qs):
                    work()

    def emit_pv(g):
        w, kc = divmod(g, KC)
        pair, qw = WINDOWS[w]
        if kc == 0:
            Umap[w] = [uacc.tile([128, 260], F32, tag=f"u{h2}",
                                 name=f"U{w}_{h2}")
                       for h2 in range(2)]
        U = Umap[w]
        pt = pts.pop(g)
        # One start=True per U bank marks the whole bank pending-zero; each
        # group's first write then zero-fills its own bytes.
        for h2 in range(2):
            h = 2 * pair + h2
            for qs in range(4):
                nc.tensor.matmul(
                    U[h2][:, qs * 65:qs * 65 + 65],
                    lhsT=pt[:, h2 * 512 + qs * 128:h2 * 512 + (qs + 1) * 128],
                    rhs=valp_ap(kc, h),
                    start=(kc == 0 and qs == 0), stop=(kc == KC - 1),
                    skip_group_check=True)
        if kc == KC - 1:
            emit_norms(w, inline_fins=(w == len(WINDOWS) - 1))

    # startup: xk0/xq0 first, then the first projection halves so their ACT
    # copies queue ahead of the bulk DMA configs, then everything else
    proj_load(0)
    pc0 = proj_chunks(0)
    pc0[0]()  # kchunk(0): kp cols 0:512
    pc0[1]()  # qchunk(0): qp window 0
    # interleave mask (qw=0 half) and valp loads roughly by deadline
    load_masks(0, [0, 1])
    load_valp([0])
    load_masks(0, [2, 3])
    load_valp([1])
    proj_load(1)
    load_masks(0, [4, 5])
    load_valp([2, 3])
    load_masks(0, [6, 7])
    load_valp([4, 5, 6, 7])
    aux = deque(pc0[2:])
    # PV slot schedule: PV(kc) trails its scores by SKEW slots; the last 8
    # PVs of each window are compressed 2-per-slot so the window's normalize
    # lands early in the next window, ahead of that window's DVE mask queue.
    from collections import defaultdict
    pv_sched = defaultdict(list)
    for g in range(NG):
        w, kc = divmod(g, KC)
        s = g + SKEW if kc < KC - 8 else w * KC + KC + (kc - (KC - 8)) // 2
        pv_sched[s].append(g)
    last_slot = max(pv_sched)
    for g in range(max(NG, last_slot + 1)):
        if g == 14:
            aux.extend(proj_chunks(1))
        elif g == 20:
            load_masks(1, range(8))
        elif g == 30:
            load_late_consts()
        elif g == 44:
            proj_load(2)
        elif g == 56:
            aux.extend(proj_chunks(2))
        elif g == 108:
            proj_load(3)
        elif g == 120:
            aux.extend(proj_chunks(3))
        if g < NG:
            emit_scores(g)
        for gpv in pv_sched.get(g, ()):
            emit_pv(gpv)
            wv, kcv = divmod(gpv, KC)
            if kcv == KC - 1 and WINDOWS[wv] == (3, 0):
                for c in range(0, 4):
                    aux.extend(finish_chunk(c))
        if aux:
            aux.popleft()()
            if aux and g >= NG - 8:
                aux.popleft()()
    while aux:
        aux.popleft()()

    ctx.close()


def _prep_inputs(key, query, value, mask, Wq, Wk, Wv, Wo, bo):
    bf16 = ml_dtypes.bfloat16
    key = np.asarray(key, np.float32)
    query = np.asarray(query, np.float32)
    value = np.asarray(value, np.float32)
    mask = np.asarray(mask)
    Wv = np.asarray(Wv, np.float32)
    Wo = np.asarray(Wo, np.float32)
    # fold the V projection into the output projection:
    # concat_h(ctxraw_h @ Wv^T) @ Wo^T == concat_raw @ (Wo @ blockdiag(Wv))^T
    Wof = np.empty_like(Wo)
    for h in range(H):
        Wof[:, h * DH:(h + 1) * DH] = Wo[:, h * DH:(h + 1) * DH] @ Wv
    common = {
        "wqT": np.ascontiguousarray(np.asarray(Wq, np.float32).T).astype(bf16),
        "wkT": np.ascontiguousarray(np.asarray(Wk, np.float32).T).astype(bf16),
        "woT": np.ascontiguousarray(Wof.T).astype(bf16),
        "bo_b": np.ascontiguousarray(
            np.broadcast_to(np.asarray(bo, np.float32), (128, E))),
        "ident": np.eye(128, dtype=np.float32).astype(bf16),
    }
    maskT = np.ascontiguousarray(
        (mask[0, 0] != 0).astype(np.float32).T.astype(bf16))  # [k, q]
    per_b = {}
    for b in range(B):
        vp = np.ones((S, H, 65), np.float32)
        vp[:, :, :64] = value[b].reshape(S, H, DH)
        per_b[b] = {
            "xkT": np.ascontiguousarray(key[b].T).astype(bf16),
            "valp": np.ascontiguousarray(vp.reshape(S, H * 65).astype(bf16)),
            "qT": query[b].T,
        }
    in_maps = []
    for c in range(N_CORES):
        b, qs = c // 4, (c % 4) * QLEN
        in_maps.append({
            "xqT": np.ascontiguousarray(
                per_b[b]["qT"][:, qs:qs + QLEN]).astype(bf16),
            "xkT": per_b[b]["xkT"],
            "valp": per_b[b]["valp"],
            "maskT": np.ascontiguousarray(maskT[:, qs:qs + QLEN]),
            **common,
        })
    return in_maps


def get_module():
    if "nc" not in _CACHE:
        _CACHE["nc"] = _build_module()
    return _CACHE["nc"]


def kernel(key, query, value, mask, Wq, Wk, Wv, Wo, bo, **_):
    nc = get_module()
    in_maps = _prep_inputs(key, query, value, mask, Wq, Wk, Wv, Wo, bo)
    res = bass_utils.run_bass_kernel_spmd(
        nc, in_maps, core_ids=list(range(N_CORES)))
    full = np.empty((B, S, E), np.float32)
    for c in range(N_CORES):
        b, qs = c // 4, (c % 4) * QLEN
        full[b, qs:qs + QLEN, :] = res.results[c]["out"]
    return full
